# revision 1
# baseline (speedup 1.0000x reference)
"""GCN aggregation (SpMM + linear) on 8 Trainium2 NeuronCores.

out = segment_sum(feature[adj_cols] * adj_vals, adj_rows) @ W.T

Strategy (all sharding internal, no collectives):
- Destination rows are sharded contiguously across the 8 cores
  (12500 rows/core). Each core owns all edges whose destination lands
  in its shard, so the segment-sum is core-local.
- Edges are bucketed by source-node window (4 windows of 25000 rows so
  gather indices fit int16) and by 128-row destination block, then
  padded to 128-edge tiles. The tile structure is made identical across
  cores (max over cores per (block, window)) because the NEFF is SPMD.
- Per tile: dma_gather pulls the 128 source rows (256B each) from a
  bf16-padded feature table in HBM straight into SBUF partitions;
  the DVE builds a val-scaled one-hot [128 edges, 128 dests] with one
  tensor_scalar (iota == ldest) * val; the PE contracts over edges:
  psum[emb, dest] += msgs^T @ valhot, accumulated per (block, window).
- Per-window aggregates (bf16) are combined inside the final W matmul
  (4 accumulating matmuls per output chunk), producing out^T in PSUM;
  out^T [64, 12500] f32 is DMAed out and transposed on the host.
"""

import os
import sys
import types

import numpy as np
import ml_dtypes

# ---------------------------------------------------------------- constants
N_NODES = 100000
N_EDGES = 1600000
EMB = 64
NC = 8
NPC = N_NODES // NC            # 12500 destination rows per core
BLK = 128                      # destination block (one-hot width)
NBLK = (NPC + BLK - 1) // BLK  # 98 blocks (last has 84 rows)
NWIN = 4
WSZ = N_NODES // NWIN          # 25000 source rows per window (< 2^15)
PAD = 128                      # feature row padded to 128 bf16 = 256B
CT_SLOTS = 1024                # max gather-chunk size: the dma_gather Q7
# ucode caps one instruction at 1024 indices (1025+ crashes the exec unit).
CORE_IDS = list(range(NC))

LAST_EXEC_NS = None            # filled when GCN_TRACE=1

_BF16 = ml_dtypes.bfloat16


# ------------------------------------------------------------- env plumbing
def _install_axon_ntff_shim():
    """bass_utils' axon trace path imports antenv.axon_hooks, which the
    container image lacks; wire it to the ctypes hook in trn_agent_boot."""
    if "antenv.axon_hooks" in sys.modules:
        return
    try:
        import trn_agent_boot.trn_boot as tb

        hook = tb._ntff_profile_via_ctypes("/opt/axon/libaxon_pjrt.so")
    except Exception:
        hook = None
    mod = types.ModuleType("antenv.axon_hooks")
    mod.get_axon_ntff_profile_hook = lambda: hook
    import antenv  # noqa: F401  (package must exist for submodule resolution)

    sys.modules["antenv.axon_hooks"] = mod


def _split_excess_waits(nc):
    """This walrus build allows at most ONE sync wait per instruction.
    Tile's scheduler freely attaches several; hoist the excess onto NoOp
    wait-carriers inserted just before the instruction (same engine, so
    engine program order preserves the blocking semantics)."""
    import bass_rust
    import concourse.mybir as mybir

    for f in nc.m.functions:
        for bb in f.blocks:
            new = []
            dirty = False
            for ins in bb.instructions:
                si = ins.sync_info
                if si is not None and len(si.on_wait) > 1:
                    waits = list(si.on_wait)
                    for k, w in enumerate(waits[:-1]):
                        nop = mybir.InstNoOp(
                            name=f"{ins.name}-pw{k}", ins=[], outs=[]
                        )
                        nop.engine = ins.engine
                        nop.sync_info = bass_rust.SyncInfo(
                            on_wait=[w], on_update=[]
                        )
                        new.append(nop)
                    si.on_wait = waits[-1:]
                    dirty = True
                new.append(ins)
            if dirty:
                bb.instructions = new


def _patch_bacc_compile():
    """Append the wait-splitter to Bacc.compile so it runs after every
    other lowering pass (walrus allows 1 sync wait per instruction)."""
    import concourse.bacc as bacc

    if getattr(bacc.Bacc, "_gcn_split_patched", False):
        return
    orig = bacc.Bacc.compile

    def _compile(self):
        orig(self)
        _split_excess_waits(self)

    bacc.Bacc.compile = _compile
    bacc.Bacc._gcn_split_patched = True


def _patch_tile_drain():
    """This walrus build rejects >1 sync wait on an InstDrain; split the
    Tile tail-drain's waits across multiple drain instructions."""
    import bass_rust
    import concourse.tile as tile
    from concourse.vector_clock import ScopedClock

    if getattr(tile.TileContext, "_gcn_drain_patched", False):
        return

    def _patched(self, tick_clock, wait_clock):
        nc = self.nc
        drain_inst = nc.sync.drain()
        wait_clock.add_sem_waits(
            drain_inst.ins, ScopedClock({None: tick_clock.global_clock})
        )
        si = drain_inst.ins.sync_info
        waits = list(si.on_wait)
        if len(waits) > 1:
            si.on_wait = waits[:1]
            for i in range(1, len(waits)):
                d2 = nc.sync.drain()
                d2.ins.sync_info = bass_rust.SyncInfo(
                    on_wait=waits[i : i + 1], on_update=[]
                )
        nc.all_engine_barrier()
        assert self.sems is not None
        popped = nc._tile_sem_poison_stack.pop()
        assert popped is self._sem_poison
        nc.clear_and_free_semaphores(list(self.sems.allocated().values()))
        nc.all_engine_barrier()

    tile.TileContext._drain_and_barrier = _patched
    tile.TileContext._gcn_drain_patched = True


# ---------------------------------------------------------- host preprocess
def _preprocess(rows, cols, vals):
    """Build the shared SPMD tile structure and per-core slot arrays.

    Returns (meta, per_core) where
      meta: dict with stream (list of (w, b, T, tile_offset)), n_tiles,
            chunks (list of (w, t0, t1)), first_w[b], block_written[b]
      per_core: list of dicts with idx16 [128, S/16], ld [128, NT] f32,
            val [128, NT] f32
    """
    core = rows // NPC
    lr = rows - core * NPC
    b = lr // BLK
    d = (lr - b * BLK).astype(np.float32)
    w = cols // WSZ
    lidx = (cols - w * WSZ).astype(np.int16)

    # group id per edge within its core: g = b * NWIN + w
    g = (b * NWIN + w).astype(np.int64)
    NG = NBLK * NWIN
    counts = np.zeros((NC, NG), np.int64)
    for c in range(NC):
        counts[c] = np.bincount(g[core == c], minlength=NG)
    tiles_per_g = -(-counts // 128)          # ceil
    T_g = tiles_per_g.max(axis=0)            # shared structure [NG]

    # stream order: (w, b) — window-major for contiguous gather windows
    stream = []          # (w, b, T, tile_offset)
    tile_off_g = np.zeros(NG, np.int64)
    t_acc = 0
    for wi in range(NWIN):
        for bi in range(NBLK):
            gi = bi * NWIN + wi
            T = int(T_g[gi])
            if T == 0:
                continue
            stream.append((wi, bi, T, t_acc))
            tile_off_g[gi] = t_acc
            t_acc += T
    n_tiles = t_acc
    S = n_tiles * 128

    # gather chunks: window-bounded slabs of <= CT_SLOTS slots
    chunks = []
    for wi in range(NWIN):
        wt = [s for s in stream if s[0] == wi]
        if not wt:
            continue
        t0 = wt[0][3]
        t1 = wt[-1][3] + wt[-1][2]
        t = t0
        while t < t1:
            te = min(t + CT_SLOTS // 128, t1)
            chunks.append((wi, t, te))
            t = te

    first_w = {}
    for wi, bi, T, _ in stream:
        if bi not in first_w:
            first_w[bi] = wi
    # windows written per block (for memset decisions)
    written = np.zeros((NBLK, NWIN), bool)
    for wi, bi, T, _ in stream:
        written[bi, wi] = True

    slot_base_g = tile_off_g * 128

    per_core = []
    for c in range(NC):
        m = core == c
        gc = g[m]
        order = np.argsort(gc, kind="stable")
        gs = gc[order]
        # rank within group
        grp_start = np.zeros(len(gs), np.int64)
        if len(gs):
            new = np.empty(len(gs), bool)
            new[0] = True
            new[1:] = gs[1:] != gs[:-1]
            idx_new = np.nonzero(new)[0]
            grp_start = idx_new[np.cumsum(new) - 1]
        rank = np.arange(len(gs)) - grp_start
        slot = slot_base_g[gs] + rank

        sidx = np.zeros(S, np.int16)
        sld = np.zeros(S, np.float32)
        sval = np.zeros(S, np.float32)
        sidx[slot] = lidx[m][order]
        sld[slot] = d[m][order]
        sval[slot] = vals[m][order]

        idx16 = np.ascontiguousarray(
            np.tile(sidx.reshape(S // 16, 16).T, (8, 1))
        )
        ld = np.ascontiguousarray(sld.reshape(n_tiles, 128).T)
        vl = np.ascontiguousarray(sval.reshape(n_tiles, 128).T)
        per_core.append({"idx": idx16, "ld": ld, "val": vl})

    meta = {
        "stream": stream,
        "n_tiles": n_tiles,
        "chunks": chunks,
        "first_w": first_w,
        "written": written,
    }
    return meta, per_core


# ------------------------------------------------------------- device build
def _build_nc(meta, max_chunks=None, do_compute=True, do_w=True):
    import concourse.bacc as bacc
    import concourse.mybir as mybir
    import concourse.tile as tile

    _patch_tile_drain()
    _patch_bacc_compile()

    stream = meta["stream"]
    n_tiles = meta["n_tiles"]
    chunks = meta["chunks"]
    if max_chunks is not None:
        chunks = chunks[:max_chunks]
    first_w = meta["first_w"]
    written = meta["written"]
    S = n_tiles * 128

    f32 = mybir.dt.float32
    bf16 = mybir.dt.bfloat16
    i16 = mybir.dt.int16

    nc = bacc.Bacc(None, target_bir_lowering=False, debug=False, num_swdge_queues=4)
    featbf = nc.declare_dram_parameter("featbf", [N_NODES, PAD], bf16, isOutput=False)
    idx_d = nc.declare_dram_parameter("idx", [128, S // 16], i16, isOutput=False)
    ld_d = nc.declare_dram_parameter("ld", [128, n_tiles], f32, isOutput=False)
    val_d = nc.declare_dram_parameter("val", [128, n_tiles], f32, isOutput=False)
    wt_d = nc.declare_dram_parameter("wt", [EMB, EMB], bf16, isOutput=False)
    iota_d = nc.declare_dram_parameter("iota", [128, BLK], bf16, isOutput=False)
    out_d = nc.declare_dram_parameter("out", [EMB, NPC], f32, isOutput=True)

    # tile index -> (w, b, k, K) lookup for the matmul loop
    tile_info = {}
    for wi, bi, T, t0 in stream:
        for k in range(T):
            tile_info[t0 + k] = (wi, bi, k, T)

    with tile.TileContext(nc) as tc:
        with (
            tc.tile_pool(name="consts", bufs=1) as cpool,
            tc.tile_pool(name="agg", bufs=1) as apool,
            tc.tile_pool(name="gather", bufs=8) as gpool,
            tc.tile_pool(name="vh", bufs=4) as vhpool,
            tc.tile_pool(name="ps", bufs=6, space="PSUM") as pspool,
            tc.tile_pool(name="wps", bufs=2, space="PSUM") as wpspool,
            tc.tile_pool(name="outb", bufs=2) as opool,
        ):
            iota_t = cpool.tile([128, BLK], bf16, tag="iota")
            nc.sync.dma_start(iota_t[:], iota_d[:])
            wt_t = cpool.tile([EMB, EMB], bf16, tag="wt")
            nc.sync.dma_start(wt_t[:], wt_d[:])
            ix_all = cpool.tile([128, S // 16], i16, tag="ixall")
            nc.sync.dma_start(ix_all[:], idx_d[:])
            ld_all = cpool.tile([128, n_tiles], f32, tag="ldall")
            nc.sync.dma_start(ld_all[:], ld_d[:])
            vl_all = cpool.tile([128, n_tiles], f32, tag="vlall")
            nc.sync.dma_start(vl_all[:], val_d[:])

            aggw = []
            for wi in range(NWIN):
                a = apool.tile([EMB, NBLK * BLK], bf16, tag=f"aggw{wi}")
                aggw.append(a)
                # zero slices never written by the stream
                holes = [bi for bi in range(NBLK) if not written[bi, wi]]
                if max_chunks is not None or not do_compute:
                    holes = list(range(NBLK))
                if len(holes) == NBLK:
                    nc.vector.memset(a[:], 0.0)
                else:
                    for bi in holes:
                        nc.vector.memset(a[:, bi * BLK : (bi + 1) * BLK], 0.0)

            psum_cur = None
            for ci, (wi, t0, t1) in enumerate(chunks):
                ctiles = t1 - t0
                cs = ctiles * 128
                g = gpool.tile([128, ctiles, PAD], bf16, tag="g")
                nc.gpsimd.dma_gather(
                    g[:, :, :],
                    featbf[wi * WSZ : (wi + 1) * WSZ, :],
                    ix_all[:, t0 * 8 : t0 * 8 + cs // 16],
                    num_idxs=cs,
                    num_idxs_reg=cs,
                    elem_size=PAD,
                    queue_num=ci % 4,
                )

                for t in range(t0, t1):
                    if not do_compute:
                        break
                    twi, bi, k, K = tile_info[t]
                    vh = vhpool.tile([128, BLK], bf16, tag="vh")
                    nc.vector.tensor_scalar(
                        vh[:],
                        iota_t[:],
                        ld_all[:, t : t + 1],
                        vl_all[:, t : t + 1],
                        mybir.AluOpType.is_equal,
                        mybir.AluOpType.mult,
                    )
                    if k == 0:
                        psum_cur = pspool.tile([EMB, BLK], f32, tag="ps")
                    nc.tensor.matmul(
                        psum_cur[:],
                        g[:, t - t0, 0:EMB],
                        vh[:],
                        start=(k == 0),
                        stop=(k == K - 1),
                    )
                    if k == K - 1:
                        nc.scalar.activation(
                            aggw[twi][:, bi * BLK : (bi + 1) * BLK],
                            psum_cur[:],
                            mybir.ActivationFunctionType.Copy,
                        )

            # final W transform: out^T[o, dest] = sum_w W.T^T @ aggw[w]
            CH = 512
            pos = 0
            while do_w and pos < NPC:
                ch = min(CH, NPC - pos)
                wps = wpspool.tile([EMB, CH], f32, tag="wps")
                for wi in range(NWIN):
                    nc.tensor.matmul(
                        wps[:, 0:ch],
                        wt_t[:],
                        aggw[wi][:, pos : pos + ch],
                        start=(wi == 0),
                        stop=(wi == NWIN - 1),
                    )
                ob = opool.tile([EMB, CH], f32, tag="ob")
                nc.scalar.activation(
                    ob[:, 0:ch], wps[:, 0:ch], mybir.ActivationFunctionType.Copy
                )
                nc.sync.dma_start(out_d[:, pos : pos + ch], ob[:, 0:ch])
                pos += ch

    nc.finalize()
    return nc


# --------------------------------------------------------------- entrypoint
def kernel(adj_rows, adj_cols, adj_vals, feature, W):
    global LAST_EXEC_NS
    _install_axon_ntff_shim()

    rows = np.asarray(adj_rows).astype(np.int64)
    cols = np.asarray(adj_cols).astype(np.int64)
    vals = np.asarray(adj_vals, dtype=np.float32)
    feat = np.asarray(feature, dtype=np.float32)
    Wm = np.asarray(W, dtype=np.float32)

    featbf = np.zeros((N_NODES, PAD), dtype=_BF16)
    featbf[:, :EMB] = feat.astype(_BF16)
    wt = np.ascontiguousarray(Wm.T).astype(_BF16)
    iota = np.broadcast_to(
        np.arange(BLK, dtype=np.float32), (128, BLK)
    ).astype(_BF16)
    iota = np.ascontiguousarray(iota)

    meta, per_core = _preprocess(rows, cols, vals)
    nc = _build_nc(meta)

    in_maps = []
    for c in range(NC):
        in_maps.append(
            {
                "featbf": featbf,
                "idx": per_core[c]["idx"],
                "ld": per_core[c]["ld"],
                "val": per_core[c]["val"],
                "wt": wt,
                "iota": iota,
            }
        )

    from concourse.bass_utils import run_bass_kernel_spmd

    res = run_bass_kernel_spmd(nc, in_maps, CORE_IDS)
    out = np.empty((N_NODES, EMB), np.float32)
    for c in range(NC):
        out[c * NPC : (c + 1) * NPC, :] = res.results[c]["out"].T

    if os.environ.get("GCN_TRACE") == "1":
        res2 = run_bass_kernel_spmd(nc, in_maps, CORE_IDS, trace=True)
        LAST_EXEC_NS = res2.exec_time_ns

    return out



# revision 2
# speedup vs baseline: 2.1270x; 2.1270x over previous
"""GCN aggregation (SpMM + linear) on 8 Trainium2 NeuronCores.

out = segment_sum(feature[adj_cols] * adj_vals, adj_rows) @ W.T

Strategy (all sharding internal, no collectives):
- Destination rows sharded contiguously across 8 cores (12500 rows/core);
  each core owns all edges whose destination lands in its shard, so the
  segment-sum is core-local. The feature table is replicated (bf16,
  rows padded to 256B so dma_gather can fetch one row per descriptor).
- Edges are bucketed by (dst 128-block, src 25000-window) groups — the
  window keeps gather indices within int16 range — and laid out in
  block-major group order. The group/tile structure is the max over
  cores (SPMD shared NEFF); per-core padding inside each group is
  trailing and marked idx=-1.
- One dma_gather per <=8-tile span of a single group. A per-core valid
  count is loaded into a GpSimd register (reg_load) right before each
  gather so trailing -1 slots generate NO DMA descriptors: descriptor
  count == actual per-core edge count (~200k), which is the wall
  (~52ns/descriptor/engine on 16 DMA engines).
- The val-scaled destination one-hot matrices ([128 edges, 128 dsts],
  one nonzero per row) are precomputed on the host from (adj_rows,
  adj_vals) and streamed bf16 — building them on the DVE serializes
  against the gather path (shared GpSimd/DVE SBUF port), which was the
  main bottleneck of the previous version.
- Per tile the PE contracts over edges: psum[emb, dst] += g^T @ onehot,
  accumulating one PSUM bank per dst block across its (<=4) groups.
  When a block completes, it is cast to bf16, multiplied by W.T on the
  PE, and DMAed out — fully fused, no aggregation buffer, no tail.
"""

import os
import sys
import types

import numpy as np
import ml_dtypes

# ---------------------------------------------------------------- constants
N_NODES = 100000
N_EDGES = 1600000
EMB = 64
NC = 8
NPC = N_NODES // NC            # 12500 destination rows per core
BLK = 128                      # destination block (one-hot width)
NBLK = (NPC + BLK - 1) // BLK  # 98 blocks per core
NWIN = 4
WSZ = N_NODES // NWIN          # 25000 source rows per window (< 2^15)
PAD = 128                      # feature row padded to 128 bf16 = 256B
MAX_CT = 8                     # max tiles per gather chunk (1024-idx ucode cap)
CORE_IDS = list(range(NC))

LAST_EXEC_NS = None            # filled when GCN_TRACE=1

_BF16 = ml_dtypes.bfloat16


# ------------------------------------------------------------- env plumbing
def _install_axon_ntff_shim():
    """bass_utils' axon trace path imports antenv.axon_hooks, which the
    container image lacks; wire it to the ctypes hook in trn_agent_boot."""
    if "antenv.axon_hooks" in sys.modules:
        return
    try:
        import trn_agent_boot.trn_boot as tb

        hook = tb._ntff_profile_via_ctypes("/opt/axon/libaxon_pjrt.so")
    except Exception:
        hook = None
    mod = types.ModuleType("antenv.axon_hooks")
    mod.get_axon_ntff_profile_hook = lambda: hook
    import antenv  # noqa: F401  (package must exist for submodule resolution)

    sys.modules["antenv.axon_hooks"] = mod


def _split_excess_waits(nc):
    """This walrus build allows at most ONE sync wait per instruction.
    Tile's scheduler freely attaches several; hoist the excess onto NoOp
    wait-carriers inserted just before the instruction (same engine, so
    engine program order preserves the blocking semantics)."""
    import bass_rust
    import concourse.mybir as mybir

    for f in nc.m.functions:
        for bb in f.blocks:
            new = []
            dirty = False
            for ins in bb.instructions:
                si = ins.sync_info
                if si is not None and len(si.on_wait) > 1:
                    waits = list(si.on_wait)
                    for k, w in enumerate(waits[:-1]):
                        nop = mybir.InstNoOp(
                            name=f"{ins.name}-pw{k}", ins=[], outs=[]
                        )
                        nop.engine = ins.engine
                        nop.sync_info = bass_rust.SyncInfo(
                            on_wait=[w], on_update=[]
                        )
                        new.append(nop)
                    si.on_wait = waits[-1:]
                    dirty = True
                new.append(ins)
            if dirty:
                bb.instructions = new


def _patch_bacc_compile():
    """Append the wait-splitter to Bacc.compile so it runs after every
    other lowering pass (walrus allows 1 sync wait per instruction)."""
    import concourse.bacc as bacc

    if getattr(bacc.Bacc, "_gcn_split_patched", False):
        return
    orig = bacc.Bacc.compile

    def _compile(self):
        orig(self)
        _split_excess_waits(self)

    bacc.Bacc.compile = _compile
    bacc.Bacc._gcn_split_patched = True


def _patch_tile_drain():
    """This walrus build rejects >1 sync wait on an InstDrain; split the
    Tile tail-drain's waits across multiple drain instructions."""
    import bass_rust
    import concourse.tile as tile
    from concourse.vector_clock import ScopedClock

    if getattr(tile.TileContext, "_gcn_drain_patched", False):
        return

    def _patched(self, tick_clock, wait_clock):
        nc = self.nc
        drain_inst = nc.sync.drain()
        wait_clock.add_sem_waits(
            drain_inst.ins, ScopedClock({None: tick_clock.global_clock})
        )
        si = drain_inst.ins.sync_info
        waits = list(si.on_wait)
        if len(waits) > 1:
            si.on_wait = waits[:1]
            for i in range(1, len(waits)):
                d2 = nc.sync.drain()
                d2.ins.sync_info = bass_rust.SyncInfo(
                    on_wait=waits[i : i + 1], on_update=[]
                )
        nc.all_engine_barrier()
        assert self.sems is not None
        popped = nc._tile_sem_poison_stack.pop()
        assert popped is self._sem_poison
        nc.clear_and_free_semaphores(list(self.sems.allocated().values()))
        nc.all_engine_barrier()

    tile.TileContext._drain_and_barrier = _patched
    tile.TileContext._gcn_drain_patched = True


# ---------------------------------------------------------- host preprocess
def _preprocess(rows, cols, vals):
    """Block-major (dst-block, window) group structure with trailing -1
    padding and per-core per-chunk valid descriptor counts.

    Returns (meta, per_core):
      meta: stream [(wi,bi,T,t0)], n_tiles, chunks [(wi,t0,t1,gi)],
            n_chunks, blk_span {bi: (first_tile, last_tile_excl)}
      per_core: idx16 [128, S/16] i16 (gather layout, -1 padding),
            cnt [1, n_chunks] i32, vhs [128, n_tiles*BLK] bf16
            (val-scaled dst one-hot rows; zero rows on padding).
    """
    core = rows // NPC
    lr = rows - core * NPC
    b = lr // BLK
    d = (lr - b * BLK).astype(np.int32)
    w = cols // WSZ
    lidx = (cols - w * WSZ).astype(np.int16)

    g = (b * NWIN + w).astype(np.int64)
    NG = NBLK * NWIN
    counts = np.zeros((NC, NG), np.int64)
    for c in range(NC):
        counts[c] = np.bincount(g[core == c], minlength=NG)
    T_g = (-(-counts // 128)).max(axis=0)

    stream = []
    tile_off_g = np.zeros(NG, np.int64)
    t_acc = 0
    for bi in range(NBLK):
        for wi in range(NWIN):
            gi = bi * NWIN + wi
            T = int(T_g[gi])
            if T == 0:
                continue
            stream.append((wi, bi, T, t_acc))
            tile_off_g[gi] = t_acc
            t_acc += T
    n_tiles = t_acc
    S = n_tiles * 128

    chunks = []
    for wi, bi, T, t0 in stream:
        gi = bi * NWIN + wi
        t = t0
        while t < t0 + T:
            te = min(t + MAX_CT, t0 + T)
            chunks.append((wi, t, te, gi))
            t = te
    n_chunks = len(chunks)

    blk_span = {}
    for wi, bi, T, t0 in stream:
        lo, hi = blk_span.get(bi, (t0, t0 + T))
        blk_span[bi] = (min(lo, t0), max(hi, t0 + T))

    slot_base_g = tile_off_g * 128

    per_core = []
    for c in range(NC):
        m = core == c
        gc = g[m]
        order = np.argsort(gc, kind="stable")
        gs = gc[order]
        grp_start = np.zeros(len(gs), np.int64)
        if len(gs):
            new = np.empty(len(gs), bool)
            new[0] = True
            new[1:] = gs[1:] != gs[:-1]
            idx_new = np.nonzero(new)[0]
            grp_start = idx_new[np.cumsum(new) - 1]
        rank = np.arange(len(gs)) - grp_start
        slot = slot_base_g[gs] + rank

        sidx = np.full(S, -1, np.int16)
        sld = np.zeros(S, np.int32)
        sval = np.zeros(S, np.float32)
        sidx[slot] = lidx[m][order]
        sld[slot] = d[m][order]
        sval[slot] = vals[m][order]

        cnt = np.zeros(n_chunks, np.int32)
        for ci, (wi, t0, t1, gi) in enumerate(chunks):
            gt0 = tile_off_g[gi]
            v = int(
                np.clip(counts[c, gi] - (t0 - gt0) * 128, 0, (t1 - t0) * 128)
            )
            if v == 0:
                # dma_gather needs >=1 valid index; fetch row 0, one-hot
                # row stays zero so it contributes nothing.
                sidx[t0 * 128] = 0
                v = 1
            cnt[ci] = v

        idx16 = np.ascontiguousarray(
            np.tile(sidx.reshape(S // 16, 16).T, (8, 1))
        )

        ld = sld.reshape(n_tiles, 128).T          # [128, n_tiles]
        vl = sval.reshape(n_tiles, 128).T
        vhs = np.zeros((128, n_tiles, BLK), dtype=_BF16)
        P, T = np.mgrid[0:128, 0:n_tiles]
        vhs[P, T, ld] = vl.astype(_BF16)

        per_core.append(
            {
                "idx": idx16,
                "cnt": cnt.reshape(1, n_chunks),
                "vhs": vhs.reshape(128, n_tiles * BLK),
            }
        )

    meta = {
        "stream": stream,
        "n_tiles": n_tiles,
        "chunks": chunks,
        "n_chunks": n_chunks,
        "blk_span": blk_span,
    }
    return meta, per_core


# ------------------------------------------------------------- device build
def _build_nc(meta, gbufs=8, vhbufs=4, psbufs=6, nq=4):
    import concourse.bacc as bacc
    import concourse.mybir as mybir
    import concourse.tile as tile

    _patch_tile_drain()
    _patch_bacc_compile()

    stream = meta["stream"]
    n_tiles = meta["n_tiles"]
    chunks = meta["chunks"]
    n_chunks = meta["n_chunks"]
    blk_span = meta["blk_span"]
    S = n_tiles * 128

    f32 = mybir.dt.float32
    bf16 = mybir.dt.bfloat16
    i16 = mybir.dt.int16
    i32 = mybir.dt.int32

    nc = bacc.Bacc(
        None, target_bir_lowering=False, debug=False, num_swdge_queues=nq
    )
    featbf = nc.declare_dram_parameter("featbf", [N_NODES, PAD], bf16, isOutput=False)
    idx_d = nc.declare_dram_parameter("idx", [128, S // 16], i16, isOutput=False)
    vh_d = nc.declare_dram_parameter("vhs", [128, n_tiles * BLK], bf16, isOutput=False)
    cnt_d = nc.declare_dram_parameter("cnt", [1, n_chunks], i32, isOutput=False)
    wt_d = nc.declare_dram_parameter("wt", [EMB, EMB], bf16, isOutput=False)
    out_d = nc.declare_dram_parameter("out", [EMB, NPC], f32, isOutput=True)

    # tile -> (block, first-tile-of-block?, last-tile-of-block?)
    tinfo = {}
    for wi, bi, T, t0 in stream:
        lo, hi = blk_span[bi]
        for k in range(T):
            t = t0 + k
            tinfo[t] = (bi, t == lo, t == hi - 1)

    covered = set(bi for _, bi, _, _ in stream)

    with tile.TileContext(nc) as tc:
        with (
            tc.tile_pool(name="consts", bufs=1) as cpool,
            tc.tile_pool(name="gather", bufs=gbufs) as gpool,
            tc.tile_pool(name="vh", bufs=vhbufs) as vhpool,
            tc.tile_pool(name="ps", bufs=psbufs, space="PSUM") as pspool,
            tc.tile_pool(name="wps", bufs=2, space="PSUM") as wpspool,
            tc.tile_pool(name="ab", bufs=3) as abpool,
            tc.tile_pool(name="outb", bufs=3) as opool,
        ):
            wt_t = cpool.tile([EMB, EMB], bf16, tag="wt")
            nc.sync.dma_start(wt_t[:], wt_d[:])
            cnt_t = cpool.tile([1, n_chunks], i32, tag="cnt")
            nc.sync.dma_start(cnt_t[:], cnt_d[:])
            ix_all = cpool.tile([128, S // 16], i16, tag="ixall")
            nc.sync.dma_start(ix_all[:], idx_d[:])

            missing = [bi for bi in range(NBLK) if bi not in covered]
            if missing:
                zeros = cpool.tile([EMB, BLK], f32, tag="zeros")
                nc.vector.memset(zeros[:], 0.0)
                for bi in missing:
                    w0 = bi * BLK
                    ch = min(BLK, NPC - w0)
                    nc.sync.dma_start(out_d[:, w0 : w0 + ch], zeros[:, 0:ch])

            creg = nc.gpsimd.alloc_register("cntreg")

            # First-use fill: per-core trailing -1 indices make the gather
            # skip slots, leaving whatever the buffer held. A NaN/Inf bit
            # pattern there would poison the matmul even through the
            # one-hot's zero rows (0 * NaN = NaN), so zero each pool
            # buffer once; afterwards stale content is always a finite
            # bf16 feature value.
            for _ in range(gbufs):
                gz = gpool.tile([128, MAX_CT, PAD], bf16, tag="g")
                nc.vector.memset(gz[:, :, :], 0.0)

            psum_cur = None
            for ci, (wi, t0, t1, gi) in enumerate(chunks):
                ctiles = t1 - t0
                cs = ctiles * 128
                vt = vhpool.tile([128, MAX_CT * BLK], bf16, tag="vhs")
                nc.sync.dma_start(
                    vt[:, 0 : ctiles * BLK], vh_d[:, t0 * BLK : t1 * BLK]
                )
                g = gpool.tile([128, ctiles, PAD], bf16, tag="g")
                nc.gpsimd.reg_load(creg, cnt_t[0:1, ci : ci + 1])
                nc.gpsimd.dma_gather(
                    g[:, :, :],
                    featbf[wi * WSZ : (wi + 1) * WSZ, :],
                    ix_all[:, t0 * 8 : t0 * 8 + cs // 16],
                    num_idxs=cs,
                    num_idxs_reg=creg,
                    elem_size=PAD,
                    queue_num=ci % nq,
                )

                for t in range(t0, t1):
                    bi, first, last = tinfo[t]
                    if first:
                        psum_cur = pspool.tile([EMB, BLK], f32, tag="ps")
                    nc.tensor.matmul(
                        psum_cur[:],
                        g[:, t - t0, 0:EMB],
                        vt[:, (t - t0) * BLK : (t - t0 + 1) * BLK],
                        start=first,
                        stop=last,
                    )
                    if last:
                        ab = abpool.tile([EMB, BLK], bf16, tag="ab")
                        nc.scalar.activation(
                            ab[:], psum_cur[:],
                            mybir.ActivationFunctionType.Copy,
                        )
                        wps = wpspool.tile([EMB, BLK], f32, tag="wps")
                        nc.tensor.matmul(
                            wps[:], wt_t[:], ab[:], start=True, stop=True
                        )
                        ob = opool.tile([EMB, BLK], f32, tag="ob")
                        w0 = bi * BLK
                        ch = min(BLK, NPC - w0)
                        nc.scalar.activation(
                            ob[:, 0:ch], wps[:, 0:ch],
                            mybir.ActivationFunctionType.Copy,
                        )
                        nc.sync.dma_start(out_d[:, w0 : w0 + ch], ob[:, 0:ch])

    nc.finalize()
    return nc


# --------------------------------------------------------------- entrypoint
def kernel(adj_rows, adj_cols, adj_vals, feature, W):
    global LAST_EXEC_NS
    _install_axon_ntff_shim()

    rows = np.asarray(adj_rows).astype(np.int64)
    cols = np.asarray(adj_cols).astype(np.int64)
    vals = np.asarray(adj_vals, dtype=np.float32)
    feat = np.asarray(feature, dtype=np.float32)
    Wm = np.asarray(W, dtype=np.float32)

    featbf = np.zeros((N_NODES, PAD), dtype=_BF16)
    featbf[:, :EMB] = feat.astype(_BF16)
    wt = np.ascontiguousarray(Wm.T).astype(_BF16)

    meta, per_core = _preprocess(rows, cols, vals)
    nc = _build_nc(meta)

    in_maps = []
    for c in range(NC):
        in_maps.append(
            {
                "featbf": featbf,
                "idx": per_core[c]["idx"],
                "vhs": per_core[c]["vhs"],
                "cnt": per_core[c]["cnt"],
                "wt": wt,
            }
        )

    from concourse.bass_utils import run_bass_kernel_spmd

    res = run_bass_kernel_spmd(nc, in_maps, CORE_IDS)
    out = np.empty((N_NODES, EMB), np.float32)
    for c in range(NC):
        out[c * NPC : (c + 1) * NPC, :] = res.results[c]["out"].T

    if os.environ.get("GCN_TRACE") == "1":
        res2 = run_bass_kernel_spmd(nc, in_maps, CORE_IDS, trace=True)
        LAST_EXEC_NS = res2.exec_time_ns

    return out


# revision 3
# speedup vs baseline: 2.1534x; 1.0124x over previous
"""GCN aggregation (SpMM + linear) on 8 Trainium2 NeuronCores.

out = segment_sum(feature[adj_cols] * adj_vals, adj_rows) @ W.T

Strategy (all sharding internal, no collectives):
- Destination rows sharded contiguously across 8 cores (12500 rows/core);
  each core owns all edges whose destination lands in its shard, so the
  segment-sum is core-local. The feature table is replicated (bf16,
  rows padded to 256B so dma_gather can fetch one row per descriptor).
- Edges are bucketed by (dst 128-block, src 25000-window) groups — the
  window keeps gather indices within int16 range — and laid out in
  block-major group order. The group/tile structure is the max over
  cores (SPMD shared NEFF); per-core padding inside each group is
  trailing and marked idx=-1.
- One dma_gather per <=8-tile span of a single group. A per-core valid
  count is loaded into a GpSimd register (reg_load) right before each
  gather so trailing -1 slots generate NO DMA descriptors: descriptor
  count == actual per-core edge count (~200k), which is the wall
  (~52ns/descriptor/engine on 16 DMA engines).
- The val-scaled destination one-hot matrices ([128 edges, 128 dsts],
  one nonzero per row) are precomputed on the host from (adj_rows,
  adj_vals) and streamed bf16 — building them on the DVE serializes
  against the gather path (shared GpSimd/DVE SBUF port), which was the
  main bottleneck of the previous version.
- Per tile the PE contracts over edges: psum[emb, dst] += g^T @ onehot,
  accumulating one PSUM bank per dst block across its (<=4) groups.
  When a block completes, it is cast to bf16, multiplied by W.T on the
  PE, and DMAed out — fully fused, no aggregation buffer, no tail.
"""

import os
import sys
import types

import numpy as np
import ml_dtypes

# ---------------------------------------------------------------- constants
N_NODES = 100000
N_EDGES = 1600000
EMB = 64
NC = 8
NPC = N_NODES // NC            # 12500 destination rows per core
BLK = 128                      # destination block (one-hot width)
NBLK = (NPC + BLK - 1) // BLK  # 98 blocks per core
NWIN = 4
WSZ = N_NODES // NWIN          # 25000 source rows per window (< 2^15)
PAD = 128                      # feature row padded to 128 bf16 = 256B
MAX_CT = 8                     # max tiles per gather chunk (1024-idx ucode cap)
CORE_IDS = list(range(NC))

LAST_EXEC_NS = None            # filled when GCN_TRACE=1

_BF16 = ml_dtypes.bfloat16


# ------------------------------------------------------------- env plumbing
def _install_axon_ntff_shim():
    """bass_utils' axon trace path imports antenv.axon_hooks, which the
    container image lacks; wire it to the ctypes hook in trn_agent_boot."""
    if "antenv.axon_hooks" in sys.modules:
        return
    try:
        import trn_agent_boot.trn_boot as tb

        hook = tb._ntff_profile_via_ctypes("/opt/axon/libaxon_pjrt.so")
    except Exception:
        hook = None
    mod = types.ModuleType("antenv.axon_hooks")
    mod.get_axon_ntff_profile_hook = lambda: hook
    import antenv  # noqa: F401  (package must exist for submodule resolution)

    sys.modules["antenv.axon_hooks"] = mod


def _split_excess_waits(nc):
    """This walrus build allows at most ONE sync wait per instruction.
    Tile's scheduler freely attaches several; hoist the excess onto NoOp
    wait-carriers inserted just before the instruction (same engine, so
    engine program order preserves the blocking semantics)."""
    import bass_rust
    import concourse.mybir as mybir

    for f in nc.m.functions:
        for bb in f.blocks:
            new = []
            dirty = False
            for ins in bb.instructions:
                si = ins.sync_info
                if si is not None and len(si.on_wait) > 1:
                    waits = list(si.on_wait)
                    for k, w in enumerate(waits[:-1]):
                        nop = mybir.InstNoOp(
                            name=f"{ins.name}-pw{k}", ins=[], outs=[]
                        )
                        nop.engine = ins.engine
                        nop.sync_info = bass_rust.SyncInfo(
                            on_wait=[w], on_update=[]
                        )
                        new.append(nop)
                    si.on_wait = waits[-1:]
                    dirty = True
                new.append(ins)
            if dirty:
                bb.instructions = new


def _patch_bacc_compile():
    """Append the wait-splitter to Bacc.compile so it runs after every
    other lowering pass (walrus allows 1 sync wait per instruction)."""
    import concourse.bacc as bacc

    if getattr(bacc.Bacc, "_gcn_split_patched", False):
        return
    orig = bacc.Bacc.compile

    def _compile(self):
        orig(self)
        _split_excess_waits(self)

    bacc.Bacc.compile = _compile
    bacc.Bacc._gcn_split_patched = True


def _patch_tile_drain():
    """This walrus build rejects >1 sync wait on an InstDrain; split the
    Tile tail-drain's waits across multiple drain instructions."""
    import bass_rust
    import concourse.tile as tile
    from concourse.vector_clock import ScopedClock

    if getattr(tile.TileContext, "_gcn_drain_patched", False):
        return

    def _patched(self, tick_clock, wait_clock):
        nc = self.nc
        drain_inst = nc.sync.drain()
        wait_clock.add_sem_waits(
            drain_inst.ins, ScopedClock({None: tick_clock.global_clock})
        )
        si = drain_inst.ins.sync_info
        waits = list(si.on_wait)
        if len(waits) > 1:
            si.on_wait = waits[:1]
            for i in range(1, len(waits)):
                d2 = nc.sync.drain()
                d2.ins.sync_info = bass_rust.SyncInfo(
                    on_wait=waits[i : i + 1], on_update=[]
                )
        nc.all_engine_barrier()
        assert self.sems is not None
        popped = nc._tile_sem_poison_stack.pop()
        assert popped is self._sem_poison
        nc.clear_and_free_semaphores(list(self.sems.allocated().values()))
        nc.all_engine_barrier()

    tile.TileContext._drain_and_barrier = _patched
    tile.TileContext._gcn_drain_patched = True


# ---------------------------------------------------------- host preprocess
def _preprocess(rows, cols, vals):
    """Block-major (dst-block, window) group structure with trailing -1
    padding and per-core per-chunk valid descriptor counts.

    Returns (meta, per_core):
      meta: stream [(wi,bi,T,t0)], n_tiles, chunks [(wi,t0,t1,gi)],
            n_chunks, blk_span {bi: (first_tile, last_tile_excl)}
      per_core: idx16 [128, S/16] i16 (gather layout, -1 padding),
            cnt [1, n_chunks] i32, vhs [128, n_tiles*BLK] bf16
            (val-scaled dst one-hot rows; zero rows on padding).
    """
    core = rows // NPC
    lr = rows - core * NPC
    b = lr // BLK
    d = (lr - b * BLK).astype(np.int32)
    w = cols // WSZ
    lidx = (cols - w * WSZ).astype(np.int16)

    g = (b * NWIN + w).astype(np.int64)
    NG = NBLK * NWIN
    counts = np.zeros((NC, NG), np.int64)
    for c in range(NC):
        counts[c] = np.bincount(g[core == c], minlength=NG)
    T_g = (-(-counts // 128)).max(axis=0)

    stream = []
    tile_off_g = np.zeros(NG, np.int64)
    t_acc = 0
    for bi in range(NBLK):
        for wi in range(NWIN):
            gi = bi * NWIN + wi
            T = int(T_g[gi])
            if T == 0:
                continue
            stream.append((wi, bi, T, t_acc))
            tile_off_g[gi] = t_acc
            t_acc += T
    n_tiles = t_acc
    S = n_tiles * 128

    chunks = []
    for wi, bi, T, t0 in stream:
        gi = bi * NWIN + wi
        t = t0
        while t < t0 + T:
            te = min(t + MAX_CT, t0 + T)
            chunks.append((wi, t, te, gi))
            t = te
    n_chunks = len(chunks)

    blk_span = {}
    for wi, bi, T, t0 in stream:
        lo, hi = blk_span.get(bi, (t0, t0 + T))
        blk_span[bi] = (min(lo, t0), max(hi, t0 + T))

    slot_base_g = tile_off_g * 128

    per_core = []
    for c in range(NC):
        m = core == c
        gc = g[m]
        order = np.argsort(gc, kind="stable")
        gs = gc[order]
        grp_start = np.zeros(len(gs), np.int64)
        if len(gs):
            new = np.empty(len(gs), bool)
            new[0] = True
            new[1:] = gs[1:] != gs[:-1]
            idx_new = np.nonzero(new)[0]
            grp_start = idx_new[np.cumsum(new) - 1]
        rank = np.arange(len(gs)) - grp_start
        slot = slot_base_g[gs] + rank

        sidx = np.full(S, -1, np.int16)
        sld = np.zeros(S, np.int32)
        sval = np.zeros(S, np.float32)
        sidx[slot] = lidx[m][order]
        sld[slot] = d[m][order]
        sval[slot] = vals[m][order]

        cnt = np.zeros(n_chunks, np.int32)
        for ci, (wi, t0, t1, gi) in enumerate(chunks):
            gt0 = tile_off_g[gi]
            v = int(
                np.clip(counts[c, gi] - (t0 - gt0) * 128, 0, (t1 - t0) * 128)
            )
            # round the valid count up to a multiple of 16 (min 16): the
            # dma_gather descriptor/sem accounting assumes 16-engine
            # spreads, and tiny or odd counts desynchronize it. Dummy
            # slots fetch row 0; their one-hot rows are zero.
            v16 = min((v + 15) // 16 * 16, (t1 - t0) * 128)
            v16 = max(v16, 16)
            if v16 > v:
                sidx[t0 * 128 + v : t0 * 128 + v16] = 0
            cnt[ci] = v16

        idx16 = np.ascontiguousarray(
            np.tile(sidx.reshape(S // 16, 16).T, (8, 1))
        )

        ld = sld.reshape(n_tiles, 128).T          # [128, n_tiles]
        vl = sval.reshape(n_tiles, 128).T
        vhs = np.zeros((128, n_tiles, BLK), dtype=_BF16)
        P, T = np.mgrid[0:128, 0:n_tiles]
        vhs[P, T, ld] = vl.astype(_BF16)

        per_core.append(
            {
                "idx": idx16,
                "cnt": cnt.reshape(1, n_chunks),
                "vhs": vhs.reshape(128, n_tiles * BLK),
            }
        )

    meta = {
        "stream": stream,
        "n_tiles": n_tiles,
        "chunks": chunks,
        "n_chunks": n_chunks,
        "blk_span": blk_span,
    }
    return meta, per_core


# ------------------------------------------------------------- device build
def _build_nc(meta, gbufs=8, vhbufs=4, psbufs=6, nq=4):
    import concourse.bacc as bacc
    import concourse.mybir as mybir
    import concourse.tile as tile

    _patch_tile_drain()
    _patch_bacc_compile()

    stream = meta["stream"]
    n_tiles = meta["n_tiles"]
    chunks = meta["chunks"]
    n_chunks = meta["n_chunks"]
    blk_span = meta["blk_span"]
    S = n_tiles * 128

    f32 = mybir.dt.float32
    bf16 = mybir.dt.bfloat16
    i16 = mybir.dt.int16
    i32 = mybir.dt.int32

    nc = bacc.Bacc(
        None, target_bir_lowering=False, debug=False, num_swdge_queues=nq
    )
    featbf = nc.declare_dram_parameter("featbf", [N_NODES, PAD], bf16, isOutput=False)
    idx_d = nc.declare_dram_parameter("idx", [128, S // 16], i16, isOutput=False)
    vh_d = nc.declare_dram_parameter("vhs", [128, n_tiles * BLK], bf16, isOutput=False)
    cnt_d = nc.declare_dram_parameter("cnt", [1, n_chunks], i32, isOutput=False)
    wt_d = nc.declare_dram_parameter("wt", [EMB, EMB], bf16, isOutput=False)
    out_d = nc.declare_dram_parameter("out", [EMB, NPC], f32, isOutput=True)

    # tile -> (block, first-tile-of-block?, last-tile-of-block?)
    tinfo = {}
    for wi, bi, T, t0 in stream:
        lo, hi = blk_span[bi]
        for k in range(T):
            t = t0 + k
            tinfo[t] = (bi, t == lo, t == hi - 1)

    covered = set(bi for _, bi, _, _ in stream)

    with tile.TileContext(nc) as tc:
        with (
            tc.tile_pool(name="consts", bufs=1) as cpool,
            tc.tile_pool(name="gather", bufs=gbufs) as gpool,
            tc.tile_pool(name="vh", bufs=vhbufs) as vhpool,
            tc.tile_pool(name="ps", bufs=psbufs, space="PSUM") as pspool,
            tc.tile_pool(name="wps", bufs=2, space="PSUM") as wpspool,
            tc.tile_pool(name="ab", bufs=3) as abpool,
            tc.tile_pool(name="outb", bufs=3) as opool,
        ):
            wt_t = cpool.tile([EMB, EMB], bf16, tag="wt")
            nc.sync.dma_start(wt_t[:], wt_d[:])
            cnt_t = cpool.tile([1, n_chunks], i32, tag="cnt")
            nc.sync.dma_start(cnt_t[:], cnt_d[:])
            ix_all = cpool.tile([128, S // 16], i16, tag="ixall")
            nc.sync.dma_start(ix_all[:], idx_d[:])

            missing = [bi for bi in range(NBLK) if bi not in covered]
            if missing:
                zeros = cpool.tile([EMB, BLK], f32, tag="zeros")
                nc.vector.memset(zeros[:], 0.0)
                for bi in missing:
                    w0 = bi * BLK
                    ch = min(BLK, NPC - w0)
                    nc.sync.dma_start(out_d[:, w0 : w0 + ch], zeros[:, 0:ch])

            creg = nc.gpsimd.alloc_register("cntreg")

            # First-use fill: per-core trailing -1 indices make the gather
            # skip slots, leaving whatever the buffer held. A NaN/Inf bit
            # pattern there would poison the matmul even through the
            # one-hot's zero rows (0 * NaN = NaN), so zero each pool
            # buffer once; afterwards stale content is always a finite
            # bf16 feature value.
            for _ in range(gbufs):
                gz = gpool.tile([128, MAX_CT, PAD], bf16, tag="g")
                nc.vector.memset(gz[:, :, :], 0.0)

            psum_cur = None
            for ci, (wi, t0, t1, gi) in enumerate(chunks):
                ctiles = t1 - t0
                cs = ctiles * 128
                vt = vhpool.tile([128, MAX_CT * BLK], bf16, tag="vhs")
                nc.sync.dma_start(
                    vt[:, 0 : ctiles * BLK], vh_d[:, t0 * BLK : t1 * BLK]
                )
                g = gpool.tile([128, ctiles, PAD], bf16, tag="g")
                nc.gpsimd.reg_load(creg, cnt_t[0:1, ci : ci + 1])
                nc.gpsimd.dma_gather(
                    g[:, :, :],
                    featbf[wi * WSZ : (wi + 1) * WSZ, :],
                    ix_all[:, t0 * 8 : t0 * 8 + cs // 16],
                    num_idxs=cs,
                    num_idxs_reg=creg,
                    elem_size=PAD,
                    queue_num=ci % nq,
                )

                for t in range(t0, t1):
                    bi, first, last = tinfo[t]
                    if first:
                        psum_cur = pspool.tile([EMB, BLK], f32, tag="ps")
                    nc.tensor.matmul(
                        psum_cur[:],
                        g[:, t - t0, 0:EMB],
                        vt[:, (t - t0) * BLK : (t - t0 + 1) * BLK],
                        start=first,
                        stop=last,
                    )
                    if last:
                        ab = abpool.tile([EMB, BLK], bf16, tag="ab")
                        nc.scalar.activation(
                            ab[:], psum_cur[:],
                            mybir.ActivationFunctionType.Copy,
                        )
                        wps = wpspool.tile([EMB, BLK], f32, tag="wps")
                        nc.tensor.matmul(
                            wps[:], wt_t[:], ab[:], start=True, stop=True
                        )
                        ob = opool.tile([EMB, BLK], f32, tag="ob")
                        w0 = bi * BLK
                        ch = min(BLK, NPC - w0)
                        nc.scalar.activation(
                            ob[:, 0:ch], wps[:, 0:ch],
                            mybir.ActivationFunctionType.Copy,
                        )
                        nc.sync.dma_start(out_d[:, w0 : w0 + ch], ob[:, 0:ch])

    nc.finalize()
    return nc


# --------------------------------------------------------------- entrypoint
def kernel(adj_rows, adj_cols, adj_vals, feature, W):
    global LAST_EXEC_NS
    _install_axon_ntff_shim()

    rows = np.asarray(adj_rows).astype(np.int64)
    cols = np.asarray(adj_cols).astype(np.int64)
    vals = np.asarray(adj_vals, dtype=np.float32)
    feat = np.asarray(feature, dtype=np.float32)
    Wm = np.asarray(W, dtype=np.float32)

    featbf = np.zeros((N_NODES, PAD), dtype=_BF16)
    featbf[:, :EMB] = feat.astype(_BF16)
    wt = np.ascontiguousarray(Wm.T).astype(_BF16)

    meta, per_core = _preprocess(rows, cols, vals)
    nc = _build_nc(meta)

    in_maps = []
    for c in range(NC):
        in_maps.append(
            {
                "featbf": featbf,
                "idx": per_core[c]["idx"],
                "vhs": per_core[c]["vhs"],
                "cnt": per_core[c]["cnt"],
                "wt": wt,
            }
        )

    from concourse.bass_utils import run_bass_kernel_spmd

    res = run_bass_kernel_spmd(nc, in_maps, CORE_IDS)
    out = np.empty((N_NODES, EMB), np.float32)
    for c in range(NC):
        out[c * NPC : (c + 1) * NPC, :] = res.results[c]["out"].T

    if os.environ.get("GCN_TRACE") == "1":
        res2 = run_bass_kernel_spmd(nc, in_maps, CORE_IDS, trace=True)
        LAST_EXEC_NS = res2.exec_time_ns

    return out
